# revision 46
# baseline (speedup 1.0000x reference)
"""ASPPConv (Gaussian-weighted dilated conv1d + sync BatchNorm + ReLU) on 8 Trainium2 cores.

Strategy: data-parallel over batch (B=8 -> 1 batch element per core).

Host prep (layout only): x cast to bf16, coords zero-padded to L+2*PAD,
conv weight pre-transposed to the PE-stationary [c, (k,cc,oc)*128+o] bf16 layout.

Per core:
  phase W: Gaussian tap weights wk[k,l] = exp(-||c(l+dk)-c(l)||^2 / (2 sigma^2))
           for the 8 non-center taps (center tap k=4 has wk identical 1), packed
           [96,1024] per 4096-col chunk (4 col-groups x 8 taps x 3 dims on
           partitions), 96->32 partition-sum via a small PE matmul, exp on ACT,
           stored to DRAM tile-major [NLT, 8, LT] bf16.
  phase M: per 512-col l-tile: DMA the bf16 x chunk into a padded persistent
           buffer, DMA-broadcast wk rows to [128, 8*512], DVE-multiply the 8
           shifted x windows by wk (2x-mode bf16), accumulate 36 bf16 matmuls
           into PSUM [o=128,512] (2 o-chunks; center tap streams x directly),
           evict PSUM->SBUF y (bf16) on ACT with per-channel sum accumulation,
           plus an ACT Square over PSUM accumulating per-channel sum-of-squares.
  phase S: finalize per-channel stats, ncfw AllReduce [128,4] across the 8
           cores (sync BN; a tiny dummy AllReduce early in the kernel warms the
           CC stream), compute scale/shift per channel.
  phase P: ACT Relu(scale*y + bias) / DVE affine+max0 per 2048-col chunk,
           stores fanned across queues.
"""


import numpy as np
import ml_dtypes

import concourse.bass as bass
import concourse.bacc as bacc
import concourse.tile as tile
import concourse.mybir as mybir
from concourse.bass_utils import run_bass_kernel_spmd

F32 = mybir.dt.float32
BF16 = mybir.dt.bfloat16
AF = mybir.ActivationFunctionType

B = 8
CIN = 256
COUT = 256
L = 16384
KT = 9
DIL = 6
PAD = DIL * (KT // 2)  # 24
SIGMA = float(DIL)     # 6.0
INV2S2 = 1.0 / (2.0 * SIGMA * SIGMA)
BN_EPS = 1e-5
NTOT = float(B * L)

LT = 512
NLT = L // LT          # 32
NCC = CIN // 128       # 2
NOC = COUT // 128      # 2
LPADDED = L + 2 * PAD
NTAP = KT - 1          # 8 weighted taps (center tap k=4 has wk == 1)

# W phase packing: partition groups cover contiguous quarters of L; each
# chunk processes 512 cols of every group at once, so chunk ch covers
# l-tiles {g*8 + ch} and the first conv tile only needs one small chunk
WG = 4                 # col-groups (l-quarters) on partitions
WGC = 512              # cols per group per chunk
NWCH = (L // WG) // WGC  # 8 chunks


def _ap(base, extra_offset, free_dims):
    """Custom AP on the same tensor as `base`: keep the partition dim, replace
    the free dims."""
    return bass.AP(
        tensor=base.tensor,
        offset=base.offset + extra_offset,
        ap=[list(base.ap[0])] + [list(d) for d in free_dims],
    )


def _build_program(nc, n_cores):
    x_d = nc.dram_tensor("x", [CIN, L], BF16, kind="ExternalInput")
    # host-gathered coord layouts (pure indexing): partition p = w*48+g*12+kq*3+d,
    # group g covers l in [g*L/4, (g+1)*L/4); cg holds the tap-shifted coords
    # (zero-padded at the ends), cr the center coords
    cg_d = nc.dram_tensor("cg", [96, L // WG], BF16, kind="ExternalInput")
    cr_d = nc.dram_tensor("cr", [96, L // WG], BF16, kind="ExternalInput")
    wt_d = nc.dram_tensor("wt", [128, KT * NCC * NOC * 128], BF16, kind="ExternalInput")
    g_d = nc.dram_tensor("gamma", [COUT], F32, kind="ExternalInput")
    bt_d = nc.dram_tensor("beta", [COUT], F32, kind="ExternalInput")
    o_d = nc.dram_tensor("out", [COUT, L], F32, kind="ExternalOutput")

    # 96->32 partition-sum matrix for the W phase, block structure:
    # partition p = w*48 + g*12 + kq*3 + d  ->  column w*4 + kq + g*8
    # (taps split in two stride-uniform halves w: k in {0..3} and {5..8})
    bmat = np.zeros((2 * WG * 4 * 3, WG * NTAP), dtype=np.float32)
    for w in range(2):
        for g in range(WG):
            for kq in range(4):
                for dd in range(3):
                    bmat[w * 48 + g * 12 + kq * 3 + dd, g * NTAP + w * 4 + kq] = 1.0
    b_dram = nc.inline_tensor(bmat.astype(ml_dtypes.bfloat16), name="bmat")

    from contextlib import ExitStack
    from concourse.tile_rust import add_dep_helper

    with tile.TileContext(nc) as tc, ExitStack() as stk:
        # ---------------- DRAM scratch ----------------
        dram = stk.enter_context(tc.tile_pool(name="dram", bufs=1, space="DRAM"))
        # tap weights, one dram tile per (chunk, group) so each per-tile
        # broadcast read depends only on its own producer scatter:
        # wk_dd[(ch,g)][h, ki, :] covers l-tiles g*8 + ch*2 + h
        wk_dd = {
            ch: dram.tile([WG, NTAP, LT], BF16, tag=f"wk{ch}", name=f"wk{ch}")
            for ch in range(NWCH)
        }
        ccin_d = dram.tile([128, 2 * NOC], F32, tag="ccin", name="ccin")
        ccout_d = dram.tile([128, 2 * NOC], F32, tag="ccout", name="ccout")
        dwi_d = dram.tile([128, 1], F32, tag="dwi", name="dwi")
        dwo_d = dram.tile([128, 1], F32, tag="dwo", name="dwo")

        # ---------------- persistent tiles ----------------
        pers = stk.enter_context(tc.tile_pool(name="pers", bufs=1))
        xbf = [
            pers.tile([128, LPADDED], BF16, tag=f"xbf{cc}", name=f"xbf{cc}")
            for cc in range(NCC)
        ]
        ybf = [
            pers.tile([128, L], BF16, tag=f"ybf{oc}", name=f"ybf{oc}")
            for oc in range(NOC)
        ]
        wT = pers.tile([128, KT * NCC * NOC * 128], BF16, tag="wT", name="wT")
        s1buf = pers.tile([128, NOC, NLT], F32, tag="s1buf", name="s1buf")
        s2buf = pers.tile([128, NOC, NLT], F32, tag="s2buf", name="s2buf")
        scl = pers.tile([128, NOC], F32, tag="scl", name="scl")
        shf = pers.tile([128, NOC], F32, tag="shf", name="shf")

        # warm the CC stream early so the stats AllReduce's setup cost is paid
        # while the conv loop runs (contents of dwi/dwo are don't-care)
        if n_cores > 1:
            nc.gpsimd.collective_compute(
                "AllReduce",
                mybir.AluOpType.add,
                replica_groups=[list(range(n_cores))],
                ins=[dwi_d[:].opt()],
                outs=[dwo_d[:].opt()],
            )

        for cc in range(NCC):
            nc.vector.memset(xbf[cc][:, 0:PAD], 0.0)
            nc.vector.memset(xbf[cc][:, L + PAD : L + 2 * PAD], 0.0)

        # x chunk loads: straight bf16 DMA into the padded persistent buffer
        load_insts = {}

        def load_chunk(t):
            insts = []
            for cc in range(NCC):
                di = nc.gpsimd.dma_start(
                    xbf[cc][:, PAD + t * LT : PAD + (t + 1) * LT],
                    x_d[cc * 128 : (cc + 1) * 128, t * LT : (t + 1) * LT],
                )
                insts.append(di)
            load_insts[t] = insts

        first_loads = []
        load_chunk(0)
        first_loads += load_insts[0]
        load_chunk(1)
        first_loads += load_insts[1]

        # ---------------- phases W+M interleaved ----------------
        sp = stk.enter_context(tc.tile_pool(name="sp", bufs=1))
        with (
            tc.tile_pool(name="mp", bufs=2) as mp,
            tc.tile_pool(name="psm", bufs=2, space="PSUM") as psm,
        ):
            bsb = mp.tile([96, WG * NTAP], BF16, tag="bsb", name="bsb", bufs=1)
            nc.sync.dma_start(bsb[:], b_dram[:, :])
            # coord gather in head (chunks 0-1) + tail DMAs: the head is the
            # kernel's critical path, so it goes first and alone
            HC = 2 * WGC
            c96 = mp.tile([96, NWCH * WGC], BF16, tag="c96", name="c96", bufs=1)
            cgi = nc.sync.dma_start(c96[:, 0:HC], cg_d[:, 0:HC])
            crep = mp.tile([96, NWCH * WGC], BF16, tag="crep", name="crep", bufs=1)
            cri = nc.sync.dma_start(crep[:, 0:HC], cr_d[:, 0:HC])
            cgi2 = nc.sync.dma_start(c96[:, HC:], cg_d[:, HC:])
            cri2 = nc.sync.dma_start(crep[:, HC:], cr_d[:, HC:])
            # wT is big (1.2MB) and needed a bit later than the coords: keep it
            # out of the startup HBM burst
            wti = nc.gpsimd.dma_start(wT[:], wt_d[:, :])
            add_dep_helper(wti.ins, cgi.ins, reason="defer wT behind coord gather")
            add_dep_helper(wti.ins, cri.ins, reason="defer wT behind coord gather")
            # the first x chunks too: the W chain is the head critical path
            for di in first_loads:
                add_dep_helper(di.ins, cgi.ins, reason="defer x behind coord gather")
                add_dep_helper(di.ins, cri.ins, reason="defer x behind coord gather")

            def emit_w_chunk(ch):
                """Gaussian tap weights for l-tiles {g*8 + ch}."""
                sl = slice(ch * WGC, (ch + 1) * WGC)
                diff = mp.tile([96, WGC], BF16, tag="diff", name="diff", bufs=1)
                nc.vector.tensor_sub(diff[:], c96[:, sl], crep[:, sl])
                sq = mp.tile([96, WGC], BF16, tag="sq", name="sq", bufs=2)
                nc.vector.tensor_mul(sq[:], diff[:], diff[:])
                wkch = mp.tile([WG * NTAP, WGC], BF16, tag="wkch", name="wkch", bufs=2)
                pw = psm.tile([WG * NTAP, WGC], F32, tag="pw", name="pw")
                nc.tensor.matmul(pw[:], bsb[:], sq[:])
                nc.scalar.activation(
                    out=wkch[:], in_=pw[:], func=AF.Exp, scale=-INV2S2
                )
                # one contiguous scatter per chunk: wk_dd[ch][g, ki, :]
                nc.scalar.dma_start(wk_dd[ch][:, :, :], wkch[:])

            # all W chunks up front at high priority: the W DVE ops are cheap
            # (16 x ~0.3us) and must not interleave behind xw ops on the DVE
            # FIFO, which would couple the scatter->broadcast round trip into
            # the W pipeline
            with tc.high_priority():
                for ch in range(NWCH):
                    emit_w_chunk(ch)
            xw_insts = {}
            for t in range(NLT):
                if t + 2 < NLT:
                    load_chunk(t + 2)
                    # pace the x prefetch: keep the startup HBM burst small so
                    # the W coord gather isn't starved for bandwidth
                    if t - 2 in xw_insts:
                        for di in load_insts[t + 2]:
                            add_dep_helper(
                                di.ins, xw_insts[t - 2].ins, reason="x prefetch pacing"
                            )
                wkb = mp.tile([128, NTAP * LT], BF16, tag="wkb", name="wkb")
                wkt = wk_dd[t % 8]
                src = bass.AP(
                    tensor=wkt.tensor,
                    offset=wkt.offset + (t // 8) * NTAP * LT,
                    ap=[[0, 128], [1, NTAP * LT]],
                )
                nc.sync.dma_start(wkb[:], src)
                xw = [
                    mp.tile([128, NTAP, LT], BF16, tag=f"xw{cc}", name=f"xw{cc}")
                    for cc in range(NCC)
                ]
                for cc in range(NCC):
                    for w in range(2):
                        src0 = _ap(
                            xbf[cc][:], t * LT + w * 5 * DIL, [[DIL, 4], [1, LT]]
                        )
                        wkv = _ap(wkb[:], w * 4 * LT, [[LT, 4], [1, LT]])
                        tt = nc.vector.tensor_mul(
                            xw[cc][:, w * 4 : (w + 1) * 4, :], src0, wkv
                        )
                        xw_insts[t] = tt
                        # windows read 24 cols into chunk t+1 (and start in
                        # chunk t-1); pin the RAW deps explicitly
                        for dep_t in (t - 1, t, t + 1):
                            for di in load_insts.get(dep_t, []):
                                add_dep_helper(
                                    tt.ins, di.ins, reason="x window chunk overlap"
                                )
                ps = [
                    psm.tile([128, LT], F32, tag=f"ps{oc}", name=f"ps{oc}")
                    for oc in range(NOC)
                ]
                for cc in range(NCC):
                    for k in range(KT):
                        if k == 4:
                            rhs = _ap(xbf[cc][:], t * LT + 4 * DIL, [[1, LT]])
                        else:
                            ki = k if k < 4 else k - 1
                            rhs = xw[cc][:, ki, :]
                        for oc in range(NOC):
                            idx = (k * NCC + cc) * NOC + oc
                            mm = nc.tensor.matmul(
                                ps[oc][:],
                                wT[:, idx * 128 : (idx + 1) * 128],
                                rhs,
                                start=(cc == 0 and k == 0),
                                stop=(cc == NCC - 1 and k == KT - 1),
                            )
                            if k == 4:
                                for di in load_insts.get(t, []):
                                    add_dep_helper(
                                        mm.ins, di.ins, reason="center tap x read"
                                    )
                sqd = psm.tile([128, LT], F32, tag="sqd", name="sqd", bufs=1)
                for oc in range(NOC):
                    nc.scalar.activation(
                        out=ybf[oc][:, t * LT : (t + 1) * LT],
                        in_=ps[oc][:],
                        func=AF.Copy,
                        accum_out=s1buf[:, oc, t : t + 1],
                    )
                    nc.scalar.activation(
                        out=sqd[:],
                        in_=ps[oc][:],
                        func=AF.Square,
                        accum_out=s2buf[:, oc, t : t + 1],
                    )

            # ------------ phase S: stats + sync-BN allreduce ------------
            stats = sp.tile([128, 2 * NOC], F32, tag="stats", name="stats")
            for oc in range(NOC):
                nc.vector.reduce_sum(
                    stats[:, oc : oc + 1], s1buf[:, oc, :], mybir.AxisListType.X
                )
                nc.vector.reduce_sum(
                    stats[:, NOC + oc : NOC + oc + 1],
                    s2buf[:, oc, :],
                    mybir.AxisListType.X,
                )
            allst = sp.tile([128, 2 * NOC], F32, tag="allst", name="allst")
            nc.sync.dma_start(ccin_d[:, :], stats[:])
            if n_cores == 1:
                # timeline-sim mode: no collectives supported; plain copy
                nc.sync.dma_start(ccout_d[:, :], ccin_d[:, :])
            else:
                nc.gpsimd.collective_compute(
                    "AllReduce",
                    mybir.AluOpType.add,
                    replica_groups=[list(range(n_cores))],
                    ins=[ccin_d[:].opt()],
                    outs=[ccout_d[:].opt()],
                )
            nc.sync.dma_start(allst[:], ccout_d[:, :])

            mean = sp.tile([128, NOC], F32, tag="mean", name="mean")
            nc.vector.tensor_scalar_mul(mean[:], allst[:, 0:NOC], 1.0 / NTOT)
            e2 = sp.tile([128, NOC], F32, tag="e2", name="e2")
            nc.vector.tensor_scalar_mul(e2[:], allst[:, NOC : 2 * NOC], 1.0 / NTOT)
            var = sp.tile([128, NOC], F32, tag="var", name="var")
            nc.vector.tensor_mul(var[:], mean[:], mean[:])
            nc.vector.tensor_sub(var[:], e2[:], var[:])
            epsc = sp.tile([128, 1], F32, tag="epsc", name="epsc")
            nc.vector.memset(epsc[:], BN_EPS)
            std = sp.tile([128, NOC], F32, tag="std", name="std")
            nc.scalar.activation(std[:], var[:], func=AF.Sqrt, bias=epsc[:])
            rstd = sp.tile([128, NOC], F32, tag="rstd", name="rstd")
            nc.vector.reciprocal(rstd[:], std[:])

            gsb = sp.tile([128, NOC], F32, tag="gsb", name="gsb")
            nc.sync.dma_start(
                gsb[:], bass.AP(tensor=g_d, offset=0, ap=[[1, 128], [128, NOC]])
            )
            btsb = sp.tile([128, NOC], F32, tag="btsb", name="btsb")
            nc.sync.dma_start(
                btsb[:], bass.AP(tensor=bt_d, offset=0, ap=[[1, 128], [128, NOC]])
            )
            nc.vector.tensor_mul(scl[:], gsb[:], rstd[:])
            tmp = sp.tile([128, NOC], F32, tag="tmp", name="tmp")
            nc.vector.tensor_mul(tmp[:], mean[:], scl[:])
            nc.vector.tensor_sub(shf[:], btsb[:], tmp[:])

        # ---------------- phase P: normalize + relu + store ----------------
        # alternate tiles between ACT (fused relu) and DVE (affine + max0) so
        # the two engines halve the compute; DMAs fan across queues
        # stores only on sync/scalar: gpsimd's teardown DRAIN is slow (~1.7us
        # each) and would gate kernel exit on its last store's completion
        PT = 2 * LT  # two l-tiles per op/DMA (0.5MB stores)
        engs = [nc.sync, nc.scalar]
        with tc.tile_pool(name="pp", bufs=8) as pp:
            i = 0
            for t in range(L // PT):
                for oc in range(NOC):
                    ot = pp.tile([128, PT], F32, tag="ot", name="ot")
                    ysl = ybf[oc][:, t * PT : (t + 1) * PT]
                    if i % 2 == 0:
                        nc.scalar.activation(
                            out=ot[:],
                            in_=ysl,
                            func=AF.Relu,
                            scale=scl[:, oc : oc + 1],
                            bias=shf[:, oc : oc + 1],
                        )
                    else:
                        nc.vector.tensor_scalar(
                            out=ot[:],
                            in0=ysl,
                            scalar1=scl[:, oc : oc + 1],
                            scalar2=shf[:, oc : oc + 1],
                            op0=mybir.AluOpType.mult,
                            op1=mybir.AluOpType.add,
                        )
                        nc.vector.tensor_scalar_max(out=ot[:], in0=ot[:], scalar1=0.0)
                    # gpsimd helps with early stores only, so its slow teardown
                    # DRAIN completes while sync/scalar finish the tail
                    if i < 20:
                        eng = [nc.sync, nc.scalar, nc.gpsimd][i % 3]
                    else:
                        eng = engs[i % 2]
                    eng.dma_start(
                        o_d[oc * 128 : (oc + 1) * 128, t * PT : (t + 1) * PT], ot[:]
                    )
                    i += 1

    return nc


_NC_CACHE = {}


def _get_nc(n_cores=B):
    if n_cores not in _NC_CACHE:
        nc = bacc.Bacc(
            "TRN2", target_bir_lowering=False, debug=False, num_devices=n_cores
        )
        _build_program(nc, n_cores)
        nc.compile()
        _NC_CACHE[n_cores] = nc
    return _NC_CACHE[n_cores]


def _install_ntff_hook():
    """The trimmed image lacks antenv.axon_hooks; synthesize it and register the
    ctypes-based NTFF profile hook so run_bass_kernel_spmd(trace=True) works."""
    import sys
    import types

    if "antenv.axon_hooks" in sys.modules:
        return
    mod = types.ModuleType("antenv.axon_hooks")
    state = {"hook": None}
    mod.set_axon_ntff_profile_hook = lambda h: state.__setitem__("hook", h)
    mod.get_axon_ntff_profile_hook = lambda: state["hook"]
    sys.modules["antenv.axon_hooks"] = mod
    try:
        from trn_agent_boot.trn_boot import _ntff_profile_via_ctypes

        mod.set_axon_ntff_profile_hook(
            _ntff_profile_via_ctypes("/opt/axon/libaxon_pjrt.so")
        )
    except Exception as e:
        print(f"ntff hook install failed: {e}")


def kernel(x, coords, weight, gamma, beta, _trace=False):
    if _trace:
        _install_ntff_hook()
    x = np.ascontiguousarray(x, dtype=np.float32)
    coords = np.ascontiguousarray(coords, dtype=np.float32)
    weight = np.ascontiguousarray(weight, dtype=np.float32)
    gamma = np.ascontiguousarray(gamma, dtype=np.float32)
    beta = np.ascontiguousarray(beta, dtype=np.float32)

    # host layout prep (pure indexing/copies): bf16 x, gathered coord layouts,
    # PE-stationary transposed weight
    x_bf = x.astype(ml_dtypes.bfloat16)
    cpad = np.zeros((B, 3, LPADDED), dtype=np.float32)
    cpad[:, :, PAD : PAD + L] = coords
    LQ = L // WG
    cg = np.empty((B, 96, LQ), dtype=np.float32)
    cr = np.empty((B, 96, LQ), dtype=np.float32)
    # bf16 coords are plenty: |wk error| <= ~0.05% given sigma=6
    for w in range(2):
        for g in range(WG):
            for kq in range(4):
                p = w * 48 + g * 12 + kq * 3
                off = g * LQ + w * 5 * DIL + kq * DIL
                cg[:, p : p + 3, :] = cpad[:, :, off : off + LQ]
                cr[:, p : p + 3, :] = cpad[:, :, PAD + g * LQ : PAD + (g + 1) * LQ]
    # wt[cw, ((k*NCC+cc)*NOC+oc)*128 + ow] = weight[oc*128+ow, cc*128+cw, k]
    wt = np.ascontiguousarray(
        weight.reshape(NOC, 128, NCC, 128, KT)
        .transpose(3, 4, 2, 0, 1)
        .reshape(128, KT * NCC * NOC * 128)
    ).astype(ml_dtypes.bfloat16)

    nc = _get_nc(B)
    in_maps = [
        {
            "x": np.ascontiguousarray(x_bf[b]),
            "cg": np.ascontiguousarray(cg[b]).astype(ml_dtypes.bfloat16),
            "cr": np.ascontiguousarray(cr[b]).astype(ml_dtypes.bfloat16),
            "wt": wt,
            "gamma": gamma,
            "beta": beta,
        }
        for b in range(B)
    ]
    res = run_bass_kernel_spmd(nc, in_maps, core_ids=list(range(B)), trace=_trace)
    out = np.stack([res.results[b]["out"] for b in range(B)], axis=0)
    if _trace:
        return out, res
    return out


# revision 47
# speedup vs baseline: 1.0157x; 1.0157x over previous
"""ASPPConv (Gaussian-weighted dilated conv1d + sync BatchNorm + ReLU) on 8 Trainium2 cores.

Strategy: data-parallel over batch (B=8 -> 1 batch element per core).

Host prep (layout only): x cast to bf16, coords zero-padded to L+2*PAD,
conv weight pre-transposed to the PE-stationary [c, (k,cc,oc)*128+o] bf16 layout.

Per core:
  phase W: Gaussian tap weights wk[k,l] = exp(-||c(l+dk)-c(l)||^2 / (2 sigma^2))
           for the 8 non-center taps (center tap k=4 has wk identical 1), packed
           [96,1024] per 4096-col chunk (4 col-groups x 8 taps x 3 dims on
           partitions), 96->32 partition-sum via a small PE matmul, exp on ACT,
           stored to DRAM tile-major [NLT, 8, LT] bf16.
  phase M: per 512-col l-tile: DMA the bf16 x chunk into a padded persistent
           buffer, DMA-broadcast wk rows to [128, 8*512], DVE-multiply the 8
           shifted x windows by wk (2x-mode bf16), accumulate 36 bf16 matmuls
           into PSUM [o=128,512] (2 o-chunks; center tap streams x directly),
           evict PSUM->SBUF y (bf16) on ACT with per-channel sum accumulation,
           plus an ACT Square over PSUM accumulating per-channel sum-of-squares.
  phase S: finalize per-channel stats, ncfw AllReduce [128,4] across the 8
           cores (sync BN; a tiny dummy AllReduce early in the kernel warms the
           CC stream), compute scale/shift per channel.
  phase P: ACT Relu(scale*y + bias) / DVE affine+max0 per 2048-col chunk,
           stores fanned across queues.
"""


import numpy as np
import ml_dtypes

import concourse.bass as bass
import concourse.bacc as bacc
import concourse.tile as tile
import concourse.mybir as mybir
from concourse.bass_utils import run_bass_kernel_spmd

F32 = mybir.dt.float32
BF16 = mybir.dt.bfloat16
AF = mybir.ActivationFunctionType

B = 8
CIN = 256
COUT = 256
L = 16384
KT = 9
DIL = 6
PAD = DIL * (KT // 2)  # 24
SIGMA = float(DIL)     # 6.0
INV2S2 = 1.0 / (2.0 * SIGMA * SIGMA)
BN_EPS = 1e-5
NTOT = float(B * L)

LT = 512
NLT = L // LT          # 32
NCC = CIN // 128       # 2
NOC = COUT // 128      # 2
LPADDED = L + 2 * PAD
NTAP = KT - 1          # 8 weighted taps (center tap k=4 has wk == 1)

# W phase packing: partition groups cover contiguous quarters of L; each
# chunk processes 512 cols of every group at once, so chunk ch covers
# l-tiles {g*8 + ch} and the first conv tile only needs one small chunk
WG = 4                 # col-groups (l-quarters) on partitions
WGC = 512              # cols per group per chunk
NWCH = (L // WG) // WGC  # 8 chunks


def _ap(base, extra_offset, free_dims):
    """Custom AP on the same tensor as `base`: keep the partition dim, replace
    the free dims."""
    return bass.AP(
        tensor=base.tensor,
        offset=base.offset + extra_offset,
        ap=[list(base.ap[0])] + [list(d) for d in free_dims],
    )


def _build_program(nc, n_cores):
    x_d = nc.dram_tensor("x", [CIN, L], BF16, kind="ExternalInput")
    # host-gathered coord layouts (pure indexing): partition p = w*48+g*12+kq*3+d,
    # group g covers l in [g*L/4, (g+1)*L/4); cg holds the tap-shifted coords
    # (zero-padded at the ends), cr the center coords
    cg_d = nc.dram_tensor("cg", [96, L // WG], BF16, kind="ExternalInput")
    cr_d = nc.dram_tensor("cr", [96, L // WG], BF16, kind="ExternalInput")
    wt_d = nc.dram_tensor("wt", [128, KT * NCC * NOC * 128], BF16, kind="ExternalInput")
    g_d = nc.dram_tensor("gamma", [COUT], F32, kind="ExternalInput")
    bt_d = nc.dram_tensor("beta", [COUT], F32, kind="ExternalInput")
    o_d = nc.dram_tensor("out", [COUT, L], F32, kind="ExternalOutput")

    # 96->32 partition-sum matrix for the W phase, block structure:
    # partition p = w*48 + g*12 + kq*3 + d  ->  column w*4 + kq + g*8
    # (taps split in two stride-uniform halves w: k in {0..3} and {5..8})
    bmat = np.zeros((2 * WG * 4 * 3, WG * NTAP), dtype=np.float32)
    for w in range(2):
        for g in range(WG):
            for kq in range(4):
                for dd in range(3):
                    bmat[w * 48 + g * 12 + kq * 3 + dd, g * NTAP + w * 4 + kq] = 1.0
    b_dram = nc.inline_tensor(bmat.astype(ml_dtypes.bfloat16), name="bmat")

    from contextlib import ExitStack
    from concourse.tile_rust import add_dep_helper

    with tile.TileContext(nc) as tc, ExitStack() as stk:
        # ---------------- DRAM scratch ----------------
        dram = stk.enter_context(tc.tile_pool(name="dram", bufs=1, space="DRAM"))
        # tap weights, one dram tile per (chunk, group) so each per-tile
        # broadcast read depends only on its own producer scatter:
        # wk_dd[(ch,g)][h, ki, :] covers l-tiles g*8 + ch*2 + h
        wk_dd = {
            ch: dram.tile([WG, NTAP, LT], BF16, tag=f"wk{ch}", name=f"wk{ch}")
            for ch in range(NWCH)
        }
        ccin_d = dram.tile([128, 2 * NOC], F32, tag="ccin", name="ccin")
        ccout_d = dram.tile([128, 2 * NOC], F32, tag="ccout", name="ccout")
        dwi_d = dram.tile([128, 1], F32, tag="dwi", name="dwi")
        dwo_d = dram.tile([128, 1], F32, tag="dwo", name="dwo")

        # ---------------- persistent tiles ----------------
        pers = stk.enter_context(tc.tile_pool(name="pers", bufs=1))
        xbf = [
            pers.tile([128, LPADDED], BF16, tag=f"xbf{cc}", name=f"xbf{cc}")
            for cc in range(NCC)
        ]
        ybf = [
            pers.tile([128, L], BF16, tag=f"ybf{oc}", name=f"ybf{oc}")
            for oc in range(NOC)
        ]
        wT = pers.tile([128, KT * NCC * NOC * 128], BF16, tag="wT", name="wT")
        s1buf = pers.tile([128, NOC, NLT], F32, tag="s1buf", name="s1buf")
        s2buf = pers.tile([128, NOC, NLT], F32, tag="s2buf", name="s2buf")
        scl = pers.tile([128, NOC], F32, tag="scl", name="scl")
        shf = pers.tile([128, NOC], F32, tag="shf", name="shf")

        # warm the CC stream early so the stats AllReduce's setup cost is paid
        # while the conv loop runs (contents of dwi/dwo are don't-care)
        if n_cores > 1:
            nc.gpsimd.collective_compute(
                "AllReduce",
                mybir.AluOpType.add,
                replica_groups=[list(range(n_cores))],
                ins=[dwi_d[:].opt()],
                outs=[dwo_d[:].opt()],
            )

        for cc in range(NCC):
            nc.vector.memset(xbf[cc][:, 0:PAD], 0.0)
            nc.vector.memset(xbf[cc][:, L + PAD : L + 2 * PAD], 0.0)

        # x chunk loads: straight bf16 DMA into the padded persistent buffer
        load_insts = {}

        def load_chunk(t):
            insts = []
            for cc in range(NCC):
                di = nc.gpsimd.dma_start(
                    xbf[cc][:, PAD + t * LT : PAD + (t + 1) * LT],
                    x_d[cc * 128 : (cc + 1) * 128, t * LT : (t + 1) * LT],
                )
                insts.append(di)
            load_insts[t] = insts

        first_loads = []
        load_chunk(0)
        first_loads += load_insts[0]
        load_chunk(1)
        first_loads += load_insts[1]

        # ---------------- phases W+M interleaved ----------------
        sp = stk.enter_context(tc.tile_pool(name="sp", bufs=1))
        with (
            tc.tile_pool(name="mp", bufs=2) as mp,
            tc.tile_pool(name="psm", bufs=2, space="PSUM") as psm,
        ):
            bsb = mp.tile([96, WG * NTAP], BF16, tag="bsb", name="bsb", bufs=1)
            nc.sync.dma_start(bsb[:], b_dram[:, :])
            # coord gather in head (chunks 0-1) + tail DMAs: the head is the
            # kernel's critical path, so it goes first and alone
            HC = 2 * WGC
            c96 = mp.tile([96, NWCH * WGC], BF16, tag="c96", name="c96", bufs=1)
            cgi = nc.sync.dma_start(c96[:, 0:HC], cg_d[:, 0:HC])
            crep = mp.tile([96, NWCH * WGC], BF16, tag="crep", name="crep", bufs=1)
            cri = nc.sync.dma_start(crep[:, 0:HC], cr_d[:, 0:HC])
            cgi2 = nc.sync.dma_start(c96[:, HC:], cg_d[:, HC:])
            cri2 = nc.sync.dma_start(crep[:, HC:], cr_d[:, HC:])
            # wT is big (1.2MB) and needed a bit later than the coords: keep it
            # out of the startup HBM burst
            wti = nc.gpsimd.dma_start(wT[:], wt_d[:, :])
            add_dep_helper(wti.ins, cgi.ins, reason="defer wT behind coord gather")
            add_dep_helper(wti.ins, cri.ins, reason="defer wT behind coord gather")
            # the first x chunks too: the W chain is the head critical path
            for di in first_loads:
                add_dep_helper(di.ins, cgi.ins, reason="defer x behind coord gather")
                add_dep_helper(di.ins, cri.ins, reason="defer x behind coord gather")

            def emit_w_chunk(ch):
                """Gaussian tap weights for l-tiles {g*8 + ch}."""
                sl = slice(ch * WGC, (ch + 1) * WGC)
                diff = mp.tile([96, WGC], BF16, tag="diff", name="diff", bufs=1)
                nc.vector.tensor_sub(diff[:], c96[:, sl], crep[:, sl])
                sq = mp.tile([96, WGC], BF16, tag="sq", name="sq", bufs=1)
                nc.vector.tensor_mul(sq[:], diff[:], diff[:])
                # 3 bufs: hide the ~2us scatter-DMA completion so the pw chain
                # runs at exp+doorbell cadence (it gates the first conv tile)
                wkch = mp.tile([WG * NTAP, WGC], BF16, tag="wkch", name="wkch", bufs=3)
                pw = psm.tile([WG * NTAP, WGC], F32, tag="pw", name="pw")
                nc.tensor.matmul(pw[:], bsb[:], sq[:])
                nc.scalar.activation(
                    out=wkch[:], in_=pw[:], func=AF.Exp, scale=-INV2S2
                )
                # one contiguous scatter per chunk: wk_dd[ch][g, ki, :]
                nc.scalar.dma_start(wk_dd[ch][:, :, :], wkch[:])

            # all W chunks up front at high priority: the W DVE ops are cheap
            # (16 x ~0.3us) and must not interleave behind xw ops on the DVE
            # FIFO, which would couple the scatter->broadcast round trip into
            # the W pipeline
            with tc.high_priority():
                for ch in range(NWCH):
                    emit_w_chunk(ch)
            xw_insts = {}
            for t in range(NLT):
                if t + 2 < NLT:
                    load_chunk(t + 2)
                    # pace the x prefetch: keep the startup HBM burst small so
                    # the W coord gather isn't starved for bandwidth
                    if t - 2 in xw_insts:
                        for di in load_insts[t + 2]:
                            add_dep_helper(
                                di.ins, xw_insts[t - 2].ins, reason="x prefetch pacing"
                            )
                wkb = mp.tile([128, NTAP * LT], BF16, tag="wkb", name="wkb")
                wkt = wk_dd[t % 8]
                src = bass.AP(
                    tensor=wkt.tensor,
                    offset=wkt.offset + (t // 8) * NTAP * LT,
                    ap=[[0, 128], [1, NTAP * LT]],
                )
                nc.sync.dma_start(wkb[:], src)
                xw = [
                    mp.tile([128, NTAP, LT], BF16, tag=f"xw{cc}", name=f"xw{cc}")
                    for cc in range(NCC)
                ]
                for cc in range(NCC):
                    for w in range(2):
                        src0 = _ap(
                            xbf[cc][:], t * LT + w * 5 * DIL, [[DIL, 4], [1, LT]]
                        )
                        wkv = _ap(wkb[:], w * 4 * LT, [[LT, 4], [1, LT]])
                        tt = nc.vector.tensor_mul(
                            xw[cc][:, w * 4 : (w + 1) * 4, :], src0, wkv
                        )
                        xw_insts[t] = tt
                        # windows read 24 cols into chunk t+1 (and start in
                        # chunk t-1); pin the RAW deps explicitly
                        for dep_t in (t - 1, t, t + 1):
                            for di in load_insts.get(dep_t, []):
                                add_dep_helper(
                                    tt.ins, di.ins, reason="x window chunk overlap"
                                )
                ps = [
                    psm.tile([128, LT], F32, tag=f"ps{oc}", name=f"ps{oc}")
                    for oc in range(NOC)
                ]
                for cc in range(NCC):
                    for k in range(KT):
                        if k == 4:
                            rhs = _ap(xbf[cc][:], t * LT + 4 * DIL, [[1, LT]])
                        else:
                            ki = k if k < 4 else k - 1
                            rhs = xw[cc][:, ki, :]
                        for oc in range(NOC):
                            idx = (k * NCC + cc) * NOC + oc
                            mm = nc.tensor.matmul(
                                ps[oc][:],
                                wT[:, idx * 128 : (idx + 1) * 128],
                                rhs,
                                start=(cc == 0 and k == 0),
                                stop=(cc == NCC - 1 and k == KT - 1),
                            )
                            if k == 4:
                                for di in load_insts.get(t, []):
                                    add_dep_helper(
                                        mm.ins, di.ins, reason="center tap x read"
                                    )
                sqd = psm.tile([128, LT], F32, tag="sqd", name="sqd", bufs=1)
                for oc in range(NOC):
                    nc.scalar.activation(
                        out=ybf[oc][:, t * LT : (t + 1) * LT],
                        in_=ps[oc][:],
                        func=AF.Copy,
                        accum_out=s1buf[:, oc, t : t + 1],
                    )
                    nc.scalar.activation(
                        out=sqd[:],
                        in_=ps[oc][:],
                        func=AF.Square,
                        accum_out=s2buf[:, oc, t : t + 1],
                    )

            # ------------ phase S: stats + sync-BN allreduce ------------
            stats = sp.tile([128, 2 * NOC], F32, tag="stats", name="stats")
            for oc in range(NOC):
                nc.vector.reduce_sum(
                    stats[:, oc : oc + 1], s1buf[:, oc, :], mybir.AxisListType.X
                )
                nc.vector.reduce_sum(
                    stats[:, NOC + oc : NOC + oc + 1],
                    s2buf[:, oc, :],
                    mybir.AxisListType.X,
                )
            allst = sp.tile([128, 2 * NOC], F32, tag="allst", name="allst")
            nc.sync.dma_start(ccin_d[:, :], stats[:])
            if n_cores == 1:
                # timeline-sim mode: no collectives supported; plain copy
                nc.sync.dma_start(ccout_d[:, :], ccin_d[:, :])
            else:
                nc.gpsimd.collective_compute(
                    "AllReduce",
                    mybir.AluOpType.add,
                    replica_groups=[list(range(n_cores))],
                    ins=[ccin_d[:].opt()],
                    outs=[ccout_d[:].opt()],
                )
            nc.sync.dma_start(allst[:], ccout_d[:, :])

            mean = sp.tile([128, NOC], F32, tag="mean", name="mean")
            nc.vector.tensor_scalar_mul(mean[:], allst[:, 0:NOC], 1.0 / NTOT)
            e2 = sp.tile([128, NOC], F32, tag="e2", name="e2")
            nc.vector.tensor_scalar_mul(e2[:], allst[:, NOC : 2 * NOC], 1.0 / NTOT)
            var = sp.tile([128, NOC], F32, tag="var", name="var")
            nc.vector.tensor_mul(var[:], mean[:], mean[:])
            nc.vector.tensor_sub(var[:], e2[:], var[:])
            epsc = sp.tile([128, 1], F32, tag="epsc", name="epsc")
            nc.vector.memset(epsc[:], BN_EPS)
            std = sp.tile([128, NOC], F32, tag="std", name="std")
            nc.scalar.activation(std[:], var[:], func=AF.Sqrt, bias=epsc[:])
            rstd = sp.tile([128, NOC], F32, tag="rstd", name="rstd")
            nc.vector.reciprocal(rstd[:], std[:])

            gsb = sp.tile([128, NOC], F32, tag="gsb", name="gsb")
            nc.sync.dma_start(
                gsb[:], bass.AP(tensor=g_d, offset=0, ap=[[1, 128], [128, NOC]])
            )
            btsb = sp.tile([128, NOC], F32, tag="btsb", name="btsb")
            nc.sync.dma_start(
                btsb[:], bass.AP(tensor=bt_d, offset=0, ap=[[1, 128], [128, NOC]])
            )
            nc.vector.tensor_mul(scl[:], gsb[:], rstd[:])
            tmp = sp.tile([128, NOC], F32, tag="tmp", name="tmp")
            nc.vector.tensor_mul(tmp[:], mean[:], scl[:])
            nc.vector.tensor_sub(shf[:], btsb[:], tmp[:])

        # ---------------- phase P: normalize + relu + store ----------------
        # alternate tiles between ACT (fused relu) and DVE (affine + max0) so
        # the two engines halve the compute; DMAs fan across queues
        # stores only on sync/scalar: gpsimd's teardown DRAIN is slow (~1.7us
        # each) and would gate kernel exit on its last store's completion
        PT = 2 * LT  # two l-tiles per op/DMA (0.5MB stores)
        engs = [nc.sync, nc.scalar]
        with tc.tile_pool(name="pp", bufs=8) as pp:
            i = 0
            for t in range(L // PT):
                for oc in range(NOC):
                    ot = pp.tile([128, PT], F32, tag="ot", name="ot")
                    ysl = ybf[oc][:, t * PT : (t + 1) * PT]
                    if i % 2 == 0:
                        nc.scalar.activation(
                            out=ot[:],
                            in_=ysl,
                            func=AF.Relu,
                            scale=scl[:, oc : oc + 1],
                            bias=shf[:, oc : oc + 1],
                        )
                    else:
                        nc.vector.tensor_scalar(
                            out=ot[:],
                            in0=ysl,
                            scalar1=scl[:, oc : oc + 1],
                            scalar2=shf[:, oc : oc + 1],
                            op0=mybir.AluOpType.mult,
                            op1=mybir.AluOpType.add,
                        )
                        nc.vector.tensor_scalar_max(out=ot[:], in0=ot[:], scalar1=0.0)
                    # gpsimd helps with early stores only, so its slow teardown
                    # DRAIN completes while sync/scalar finish the tail
                    if i < 20:
                        eng = [nc.sync, nc.scalar, nc.gpsimd][i % 3]
                    else:
                        eng = engs[i % 2]
                    eng.dma_start(
                        o_d[oc * 128 : (oc + 1) * 128, t * PT : (t + 1) * PT], ot[:]
                    )
                    i += 1

    return nc


_NC_CACHE = {}


def _get_nc(n_cores=B):
    if n_cores not in _NC_CACHE:
        nc = bacc.Bacc(
            "TRN2", target_bir_lowering=False, debug=False, num_devices=n_cores
        )
        _build_program(nc, n_cores)
        nc.compile()
        _NC_CACHE[n_cores] = nc
    return _NC_CACHE[n_cores]


def _install_ntff_hook():
    """The trimmed image lacks antenv.axon_hooks; synthesize it and register the
    ctypes-based NTFF profile hook so run_bass_kernel_spmd(trace=True) works."""
    import sys
    import types

    if "antenv.axon_hooks" in sys.modules:
        return
    mod = types.ModuleType("antenv.axon_hooks")
    state = {"hook": None}
    mod.set_axon_ntff_profile_hook = lambda h: state.__setitem__("hook", h)
    mod.get_axon_ntff_profile_hook = lambda: state["hook"]
    sys.modules["antenv.axon_hooks"] = mod
    try:
        from trn_agent_boot.trn_boot import _ntff_profile_via_ctypes

        mod.set_axon_ntff_profile_hook(
            _ntff_profile_via_ctypes("/opt/axon/libaxon_pjrt.so")
        )
    except Exception as e:
        print(f"ntff hook install failed: {e}")


def kernel(x, coords, weight, gamma, beta, _trace=False):
    if _trace:
        _install_ntff_hook()
    x = np.ascontiguousarray(x, dtype=np.float32)
    coords = np.ascontiguousarray(coords, dtype=np.float32)
    weight = np.ascontiguousarray(weight, dtype=np.float32)
    gamma = np.ascontiguousarray(gamma, dtype=np.float32)
    beta = np.ascontiguousarray(beta, dtype=np.float32)

    # host layout prep (pure indexing/copies): bf16 x, gathered coord layouts,
    # PE-stationary transposed weight
    x_bf = x.astype(ml_dtypes.bfloat16)
    cpad = np.zeros((B, 3, LPADDED), dtype=np.float32)
    cpad[:, :, PAD : PAD + L] = coords
    LQ = L // WG
    cg = np.empty((B, 96, LQ), dtype=np.float32)
    cr = np.empty((B, 96, LQ), dtype=np.float32)
    # bf16 coords are plenty: |wk error| <= ~0.05% given sigma=6
    for w in range(2):
        for g in range(WG):
            for kq in range(4):
                p = w * 48 + g * 12 + kq * 3
                off = g * LQ + w * 5 * DIL + kq * DIL
                cg[:, p : p + 3, :] = cpad[:, :, off : off + LQ]
                cr[:, p : p + 3, :] = cpad[:, :, PAD + g * LQ : PAD + (g + 1) * LQ]
    # wt[cw, ((k*NCC+cc)*NOC+oc)*128 + ow] = weight[oc*128+ow, cc*128+cw, k]
    wt = np.ascontiguousarray(
        weight.reshape(NOC, 128, NCC, 128, KT)
        .transpose(3, 4, 2, 0, 1)
        .reshape(128, KT * NCC * NOC * 128)
    ).astype(ml_dtypes.bfloat16)

    nc = _get_nc(B)
    in_maps = [
        {
            "x": np.ascontiguousarray(x_bf[b]),
            "cg": np.ascontiguousarray(cg[b]).astype(ml_dtypes.bfloat16),
            "cr": np.ascontiguousarray(cr[b]).astype(ml_dtypes.bfloat16),
            "wt": wt,
            "gamma": gamma,
            "beta": beta,
        }
        for b in range(B)
    ]
    res = run_bass_kernel_spmd(nc, in_maps, core_ids=list(range(B)), trace=_trace)
    out = np.stack([res.results[b]["out"] for b in range(B)], axis=0)
    if _trace:
        return out, res
    return out


# revision 48
# speedup vs baseline: 1.0342x; 1.0182x over previous
"""ASPPConv (Gaussian-weighted dilated conv1d + sync BatchNorm + ReLU) on 8 Trainium2 cores.

Strategy: data-parallel over batch (B=8 -> 1 batch element per core).

Host prep (layout only): x cast to bf16, coords zero-padded to L+2*PAD,
conv weight pre-transposed to the PE-stationary [c, (k,cc,oc)*128+o] bf16 layout.

Per core:
  phase W: Gaussian tap weights wk[k,l] = exp(-||c(l+dk)-c(l)||^2 / (2 sigma^2))
           for the 8 non-center taps (center tap k=4 has wk identical 1), packed
           [96,1024] per 4096-col chunk (4 col-groups x 8 taps x 3 dims on
           partitions), 96->32 partition-sum via a small PE matmul, exp on ACT,
           stored to DRAM tile-major [NLT, 8, LT] bf16.
  phase M: per 512-col l-tile: DMA the bf16 x chunk into a padded persistent
           buffer, DMA-broadcast wk rows to [128, 8*512], DVE-multiply the 8
           shifted x windows by wk (2x-mode bf16), accumulate 36 bf16 matmuls
           into PSUM [o=128,512] (2 o-chunks; center tap streams x directly),
           evict PSUM->SBUF y (bf16) on ACT with per-channel sum accumulation,
           plus an ACT Square over PSUM accumulating per-channel sum-of-squares.
  phase S: finalize per-channel stats, ncfw AllReduce [128,4] across the 8
           cores (sync BN; a tiny dummy AllReduce early in the kernel warms the
           CC stream), compute scale/shift per channel.
  phase P: ACT Relu(scale*y + bias) / DVE affine+max0 per 2048-col chunk,
           stores fanned across queues.
"""


import numpy as np
import ml_dtypes

import concourse.bass as bass
import concourse.bacc as bacc
import concourse.tile as tile
import concourse.mybir as mybir
from concourse.bass_utils import run_bass_kernel_spmd

F32 = mybir.dt.float32
BF16 = mybir.dt.bfloat16
AF = mybir.ActivationFunctionType

B = 8
CIN = 256
COUT = 256
L = 16384
KT = 9
DIL = 6
PAD = DIL * (KT // 2)  # 24
SIGMA = float(DIL)     # 6.0
INV2S2 = 1.0 / (2.0 * SIGMA * SIGMA)
BN_EPS = 1e-5
NTOT = float(B * L)

LT = 512
NLT = L // LT          # 32
NCC = CIN // 128       # 2
NOC = COUT // 128      # 2
LPADDED = L + 2 * PAD
NTAP = KT - 1          # 8 weighted taps (center tap k=4 has wk == 1)

# W phase packing: partition groups cover contiguous quarters of L; each
# chunk processes 512 cols of every group at once, so chunk ch covers
# l-tiles {g*8 + ch} and the first conv tile only needs one small chunk
WG = 4                 # col-groups (l-quarters) on partitions
WGC = 512              # cols per group per chunk
NWCH = (L // WG) // WGC  # 8 chunks


def _ap(base, extra_offset, free_dims):
    """Custom AP on the same tensor as `base`: keep the partition dim, replace
    the free dims."""
    return bass.AP(
        tensor=base.tensor,
        offset=base.offset + extra_offset,
        ap=[list(base.ap[0])] + [list(d) for d in free_dims],
    )


def _build_program(nc, n_cores):
    x_d = nc.dram_tensor("x", [CIN, L], BF16, kind="ExternalInput")
    # host-gathered coord layouts (pure indexing): partition p = w*48+g*12+kq*3+d,
    # group g covers l in [g*L/4, (g+1)*L/4); cg holds the tap-shifted coords
    # (zero-padded at the ends), cr the center coords
    cg_d = nc.dram_tensor("cg", [96, L // WG], BF16, kind="ExternalInput")
    cr_d = nc.dram_tensor("cr", [96, L // WG], BF16, kind="ExternalInput")
    wt_d = nc.dram_tensor("wt", [128, KT * NCC * NOC * 128], BF16, kind="ExternalInput")
    g_d = nc.dram_tensor("gamma", [COUT], F32, kind="ExternalInput")
    bt_d = nc.dram_tensor("beta", [COUT], F32, kind="ExternalInput")
    o_d = nc.dram_tensor("out", [COUT, L], F32, kind="ExternalOutput")

    # 96->32 partition-sum matrix for the W phase, block structure:
    # partition p = w*48 + g*12 + kq*3 + d  ->  column w*4 + kq + g*8
    # (taps split in two stride-uniform halves w: k in {0..3} and {5..8})
    bmat = np.zeros((2 * WG * 4 * 3, WG * NTAP), dtype=np.float32)
    for w in range(2):
        for g in range(WG):
            for kq in range(4):
                for dd in range(3):
                    bmat[w * 48 + g * 12 + kq * 3 + dd, g * NTAP + w * 4 + kq] = 1.0
    b_dram = nc.inline_tensor(bmat.astype(ml_dtypes.bfloat16), name="bmat")

    from contextlib import ExitStack
    from concourse.tile_rust import add_dep_helper

    with tile.TileContext(nc) as tc, ExitStack() as stk:
        # ---------------- DRAM scratch ----------------
        dram = stk.enter_context(tc.tile_pool(name="dram", bufs=1, space="DRAM"))
        # tap weights, one dram tile per (chunk, group) so each per-tile
        # broadcast read depends only on its own producer scatter:
        # wk_dd[(ch,g)][h, ki, :] covers l-tiles g*8 + ch*2 + h
        wk_dd = {
            ch: dram.tile([WG, NTAP, LT], BF16, tag=f"wk{ch}", name=f"wk{ch}")
            for ch in range(NWCH)
        }
        ccin_d = dram.tile([128, 2 * NOC], F32, tag="ccin", name="ccin")
        ccout_d = dram.tile([128, 2 * NOC], F32, tag="ccout", name="ccout")
        dwi_d = dram.tile([128, 1], F32, tag="dwi", name="dwi")
        dwo_d = dram.tile([128, 1], F32, tag="dwo", name="dwo")

        # ---------------- persistent tiles ----------------
        pers = stk.enter_context(tc.tile_pool(name="pers", bufs=1))
        xbf = [
            pers.tile([128, LPADDED], BF16, tag=f"xbf{cc}", name=f"xbf{cc}")
            for cc in range(NCC)
        ]
        ybf = [
            pers.tile([128, L], BF16, tag=f"ybf{oc}", name=f"ybf{oc}")
            for oc in range(NOC)
        ]
        wT = pers.tile([128, KT * NCC * NOC * 128], BF16, tag="wT", name="wT")
        s1buf = pers.tile([128, NOC, NLT], F32, tag="s1buf", name="s1buf")
        s2buf = pers.tile([128, NOC, NLT], F32, tag="s2buf", name="s2buf")
        scl = pers.tile([128, NOC], F32, tag="scl", name="scl")
        shf = pers.tile([128, NOC], F32, tag="shf", name="shf")

        # warm the CC stream early so the stats AllReduce's setup cost is paid
        # while the conv loop runs (contents of dwi/dwo are don't-care)
        if n_cores > 1:
            nc.gpsimd.collective_compute(
                "AllReduce",
                mybir.AluOpType.add,
                replica_groups=[list(range(n_cores))],
                ins=[dwi_d[:].opt()],
                outs=[dwo_d[:].opt()],
            )

        for cc in range(NCC):
            nc.vector.memset(xbf[cc][:, 0:PAD], 0.0)
            nc.vector.memset(xbf[cc][:, L + PAD : L + 2 * PAD], 0.0)

        # x chunk loads: straight bf16 DMA into the padded persistent buffer
        load_insts = {}

        def load_chunk(t):
            insts = []
            for cc in range(NCC):
                di = nc.gpsimd.dma_start(
                    xbf[cc][:, PAD + t * LT : PAD + (t + 1) * LT],
                    x_d[cc * 128 : (cc + 1) * 128, t * LT : (t + 1) * LT],
                )
                insts.append(di)
            load_insts[t] = insts

        first_loads = []
        load_chunk(0)
        first_loads += load_insts[0]
        load_chunk(1)
        first_loads += load_insts[1]

        # ---------------- phases W+M interleaved ----------------
        sp = stk.enter_context(tc.tile_pool(name="sp", bufs=1))
        with (
            tc.tile_pool(name="mp", bufs=2) as mp,
            tc.tile_pool(name="psm", bufs=2, space="PSUM") as psm,
        ):
            bsb = mp.tile([96, WG * NTAP], BF16, tag="bsb", name="bsb", bufs=1)
            nc.sync.dma_start(bsb[:], b_dram[:, :])
            # coord gather split finest-first: chunk 0 alone (98KB, lands
            # earliest), then chunk 1, then the rest — the chunk-0 chain is
            # the kernel's critical path
            c96 = mp.tile([96, NWCH * WGC], BF16, tag="c96", name="c96", bufs=1)
            crep = mp.tile([96, NWCH * WGC], BF16, tag="crep", name="crep", bufs=1)
            cgi = nc.sync.dma_start(c96[:, 0:WGC], cg_d[:, 0:WGC])
            cri = nc.sync.dma_start(crep[:, 0:WGC], cr_d[:, 0:WGC])
            nc.sync.dma_start(c96[:, WGC : 2 * WGC], cg_d[:, WGC : 2 * WGC])
            nc.sync.dma_start(crep[:, WGC : 2 * WGC], cr_d[:, WGC : 2 * WGC])
            nc.sync.dma_start(c96[:, 2 * WGC :], cg_d[:, 2 * WGC :])
            nc.sync.dma_start(crep[:, 2 * WGC :], cr_d[:, 2 * WGC :])
            # wT is big (1.2MB) and needed a bit later than the coords: keep it
            # out of the startup HBM burst
            wti = nc.gpsimd.dma_start(wT[:], wt_d[:, :])
            add_dep_helper(wti.ins, cgi.ins, reason="defer wT behind coord gather")
            add_dep_helper(wti.ins, cri.ins, reason="defer wT behind coord gather")
            # the first x chunks too: the W chain is the head critical path
            for di in first_loads:
                add_dep_helper(di.ins, cgi.ins, reason="defer x behind coord gather")
                add_dep_helper(di.ins, cri.ins, reason="defer x behind coord gather")

            def emit_w_chunk(ch):
                """Gaussian tap weights for l-tiles {g*8 + ch}."""
                sl = slice(ch * WGC, (ch + 1) * WGC)
                diff = mp.tile([96, WGC], BF16, tag="diff", name="diff", bufs=1)
                nc.vector.tensor_sub(diff[:], c96[:, sl], crep[:, sl])
                sq = mp.tile([96, WGC], BF16, tag="sq", name="sq", bufs=1)
                nc.vector.tensor_mul(sq[:], diff[:], diff[:])
                # 3 bufs: hide the ~2us scatter-DMA completion so the pw chain
                # runs at exp+doorbell cadence (it gates the first conv tile)
                wkch = mp.tile([WG * NTAP, WGC], BF16, tag="wkch", name="wkch", bufs=3)
                pw = psm.tile([WG * NTAP, WGC], F32, tag="pw", name="pw")
                nc.tensor.matmul(pw[:], bsb[:], sq[:])
                nc.scalar.activation(
                    out=wkch[:], in_=pw[:], func=AF.Exp, scale=-INV2S2
                )
                # one contiguous scatter per chunk: wk_dd[ch][g, ki, :]
                nc.scalar.dma_start(wk_dd[ch][:, :, :], wkch[:])

            # all W chunks up front at high priority: the W DVE ops are cheap
            # (16 x ~0.3us) and must not interleave behind xw ops on the DVE
            # FIFO, which would couple the scatter->broadcast round trip into
            # the W pipeline
            with tc.high_priority():
                for ch in range(NWCH):
                    emit_w_chunk(ch)
            xw_insts = {}
            for t in range(NLT):
                if t + 2 < NLT:
                    load_chunk(t + 2)
                    # pace the x prefetch: keep the startup HBM burst small so
                    # the W coord gather isn't starved for bandwidth
                    if t - 2 in xw_insts:
                        for di in load_insts[t + 2]:
                            add_dep_helper(
                                di.ins, xw_insts[t - 2].ins, reason="x prefetch pacing"
                            )
                wkb = mp.tile([128, NTAP * LT], BF16, tag="wkb", name="wkb")
                wkt = wk_dd[t % 8]
                src = bass.AP(
                    tensor=wkt.tensor,
                    offset=wkt.offset + (t // 8) * NTAP * LT,
                    ap=[[0, 128], [1, NTAP * LT]],
                )
                nc.sync.dma_start(wkb[:], src)
                xw = [
                    mp.tile([128, NTAP, LT], BF16, tag=f"xw{cc}", name=f"xw{cc}")
                    for cc in range(NCC)
                ]
                for cc in range(NCC):
                    for w in range(2):
                        src0 = _ap(
                            xbf[cc][:], t * LT + w * 5 * DIL, [[DIL, 4], [1, LT]]
                        )
                        wkv = _ap(wkb[:], w * 4 * LT, [[LT, 4], [1, LT]])
                        tt = nc.vector.tensor_mul(
                            xw[cc][:, w * 4 : (w + 1) * 4, :], src0, wkv
                        )
                        xw_insts[t] = tt
                        # windows read 24 cols into chunk t+1 (and start in
                        # chunk t-1); pin the RAW deps explicitly
                        for dep_t in (t - 1, t, t + 1):
                            for di in load_insts.get(dep_t, []):
                                add_dep_helper(
                                    tt.ins, di.ins, reason="x window chunk overlap"
                                )
                ps = [
                    psm.tile([128, LT], F32, tag=f"ps{oc}", name=f"ps{oc}")
                    for oc in range(NOC)
                ]
                for cc in range(NCC):
                    for k in range(KT):
                        if k == 4:
                            rhs = _ap(xbf[cc][:], t * LT + 4 * DIL, [[1, LT]])
                        else:
                            ki = k if k < 4 else k - 1
                            rhs = xw[cc][:, ki, :]
                        for oc in range(NOC):
                            idx = (k * NCC + cc) * NOC + oc
                            mm = nc.tensor.matmul(
                                ps[oc][:],
                                wT[:, idx * 128 : (idx + 1) * 128],
                                rhs,
                                start=(cc == 0 and k == 0),
                                stop=(cc == NCC - 1 and k == KT - 1),
                            )
                            if k == 4:
                                for di in load_insts.get(t, []):
                                    add_dep_helper(
                                        mm.ins, di.ins, reason="center tap x read"
                                    )
                sqd = psm.tile([128, LT], F32, tag="sqd", name="sqd", bufs=1)
                for oc in range(NOC):
                    nc.scalar.activation(
                        out=ybf[oc][:, t * LT : (t + 1) * LT],
                        in_=ps[oc][:],
                        func=AF.Copy,
                        accum_out=s1buf[:, oc, t : t + 1],
                    )
                    nc.scalar.activation(
                        out=sqd[:],
                        in_=ps[oc][:],
                        func=AF.Square,
                        accum_out=s2buf[:, oc, t : t + 1],
                    )

            # ------------ phase S: stats + sync-BN allreduce ------------
            stats = sp.tile([128, 2 * NOC], F32, tag="stats", name="stats")
            for oc in range(NOC):
                nc.vector.reduce_sum(
                    stats[:, oc : oc + 1], s1buf[:, oc, :], mybir.AxisListType.X
                )
                nc.vector.reduce_sum(
                    stats[:, NOC + oc : NOC + oc + 1],
                    s2buf[:, oc, :],
                    mybir.AxisListType.X,
                )
            allst = sp.tile([128, 2 * NOC], F32, tag="allst", name="allst")
            nc.sync.dma_start(ccin_d[:, :], stats[:])
            if n_cores == 1:
                # timeline-sim mode: no collectives supported; plain copy
                nc.sync.dma_start(ccout_d[:, :], ccin_d[:, :])
            else:
                nc.gpsimd.collective_compute(
                    "AllReduce",
                    mybir.AluOpType.add,
                    replica_groups=[list(range(n_cores))],
                    ins=[ccin_d[:].opt()],
                    outs=[ccout_d[:].opt()],
                )
            nc.sync.dma_start(allst[:], ccout_d[:, :])

            mean = sp.tile([128, NOC], F32, tag="mean", name="mean")
            nc.vector.tensor_scalar_mul(mean[:], allst[:, 0:NOC], 1.0 / NTOT)
            e2 = sp.tile([128, NOC], F32, tag="e2", name="e2")
            nc.vector.tensor_scalar_mul(e2[:], allst[:, NOC : 2 * NOC], 1.0 / NTOT)
            var = sp.tile([128, NOC], F32, tag="var", name="var")
            nc.vector.tensor_mul(var[:], mean[:], mean[:])
            nc.vector.tensor_sub(var[:], e2[:], var[:])
            epsc = sp.tile([128, 1], F32, tag="epsc", name="epsc")
            nc.vector.memset(epsc[:], BN_EPS)
            std = sp.tile([128, NOC], F32, tag="std", name="std")
            nc.scalar.activation(std[:], var[:], func=AF.Sqrt, bias=epsc[:])
            rstd = sp.tile([128, NOC], F32, tag="rstd", name="rstd")
            nc.vector.reciprocal(rstd[:], std[:])

            gsb = sp.tile([128, NOC], F32, tag="gsb", name="gsb")
            nc.sync.dma_start(
                gsb[:], bass.AP(tensor=g_d, offset=0, ap=[[1, 128], [128, NOC]])
            )
            btsb = sp.tile([128, NOC], F32, tag="btsb", name="btsb")
            nc.sync.dma_start(
                btsb[:], bass.AP(tensor=bt_d, offset=0, ap=[[1, 128], [128, NOC]])
            )
            nc.vector.tensor_mul(scl[:], gsb[:], rstd[:])
            tmp = sp.tile([128, NOC], F32, tag="tmp", name="tmp")
            nc.vector.tensor_mul(tmp[:], mean[:], scl[:])
            nc.vector.tensor_sub(shf[:], btsb[:], tmp[:])

        # ---------------- phase P: normalize + relu + store ----------------
        # alternate tiles between ACT (fused relu) and DVE (affine + max0) so
        # the two engines halve the compute; DMAs fan across queues
        # stores only on sync/scalar: gpsimd's teardown DRAIN is slow (~1.7us
        # each) and would gate kernel exit on its last store's completion
        PT = 2 * LT  # two l-tiles per op/DMA (0.5MB stores)
        engs = [nc.sync, nc.scalar]
        with tc.tile_pool(name="pp", bufs=8) as pp:
            i = 0
            for t in range(L // PT):
                for oc in range(NOC):
                    ot = pp.tile([128, PT], F32, tag="ot", name="ot")
                    ysl = ybf[oc][:, t * PT : (t + 1) * PT]
                    if i % 2 == 0:
                        nc.scalar.activation(
                            out=ot[:],
                            in_=ysl,
                            func=AF.Relu,
                            scale=scl[:, oc : oc + 1],
                            bias=shf[:, oc : oc + 1],
                        )
                    else:
                        nc.vector.tensor_scalar(
                            out=ot[:],
                            in0=ysl,
                            scalar1=scl[:, oc : oc + 1],
                            scalar2=shf[:, oc : oc + 1],
                            op0=mybir.AluOpType.mult,
                            op1=mybir.AluOpType.add,
                        )
                        nc.vector.tensor_scalar_max(out=ot[:], in0=ot[:], scalar1=0.0)
                    # gpsimd helps with early stores only, so its slow teardown
                    # DRAIN completes while sync/scalar finish the tail
                    if i < 20:
                        eng = [nc.sync, nc.scalar, nc.gpsimd][i % 3]
                    else:
                        eng = engs[i % 2]
                    eng.dma_start(
                        o_d[oc * 128 : (oc + 1) * 128, t * PT : (t + 1) * PT], ot[:]
                    )
                    i += 1

    return nc


_NC_CACHE = {}


def _get_nc(n_cores=B):
    if n_cores not in _NC_CACHE:
        nc = bacc.Bacc(
            "TRN2", target_bir_lowering=False, debug=False, num_devices=n_cores
        )
        _build_program(nc, n_cores)
        nc.compile()
        _NC_CACHE[n_cores] = nc
    return _NC_CACHE[n_cores]


def _install_ntff_hook():
    """The trimmed image lacks antenv.axon_hooks; synthesize it and register the
    ctypes-based NTFF profile hook so run_bass_kernel_spmd(trace=True) works."""
    import sys
    import types

    if "antenv.axon_hooks" in sys.modules:
        return
    mod = types.ModuleType("antenv.axon_hooks")
    state = {"hook": None}
    mod.set_axon_ntff_profile_hook = lambda h: state.__setitem__("hook", h)
    mod.get_axon_ntff_profile_hook = lambda: state["hook"]
    sys.modules["antenv.axon_hooks"] = mod
    try:
        from trn_agent_boot.trn_boot import _ntff_profile_via_ctypes

        mod.set_axon_ntff_profile_hook(
            _ntff_profile_via_ctypes("/opt/axon/libaxon_pjrt.so")
        )
    except Exception as e:
        print(f"ntff hook install failed: {e}")


def kernel(x, coords, weight, gamma, beta, _trace=False):
    if _trace:
        _install_ntff_hook()
    x = np.ascontiguousarray(x, dtype=np.float32)
    coords = np.ascontiguousarray(coords, dtype=np.float32)
    weight = np.ascontiguousarray(weight, dtype=np.float32)
    gamma = np.ascontiguousarray(gamma, dtype=np.float32)
    beta = np.ascontiguousarray(beta, dtype=np.float32)

    # host layout prep (pure indexing/copies): bf16 x, gathered coord layouts,
    # PE-stationary transposed weight
    x_bf = x.astype(ml_dtypes.bfloat16)
    cpad = np.zeros((B, 3, LPADDED), dtype=np.float32)
    cpad[:, :, PAD : PAD + L] = coords
    LQ = L // WG
    cg = np.empty((B, 96, LQ), dtype=np.float32)
    cr = np.empty((B, 96, LQ), dtype=np.float32)
    # bf16 coords are plenty: |wk error| <= ~0.05% given sigma=6
    for w in range(2):
        for g in range(WG):
            for kq in range(4):
                p = w * 48 + g * 12 + kq * 3
                off = g * LQ + w * 5 * DIL + kq * DIL
                cg[:, p : p + 3, :] = cpad[:, :, off : off + LQ]
                cr[:, p : p + 3, :] = cpad[:, :, PAD + g * LQ : PAD + (g + 1) * LQ]
    # wt[cw, ((k*NCC+cc)*NOC+oc)*128 + ow] = weight[oc*128+ow, cc*128+cw, k]
    wt = np.ascontiguousarray(
        weight.reshape(NOC, 128, NCC, 128, KT)
        .transpose(3, 4, 2, 0, 1)
        .reshape(128, KT * NCC * NOC * 128)
    ).astype(ml_dtypes.bfloat16)

    nc = _get_nc(B)
    in_maps = [
        {
            "x": np.ascontiguousarray(x_bf[b]),
            "cg": np.ascontiguousarray(cg[b]).astype(ml_dtypes.bfloat16),
            "cr": np.ascontiguousarray(cr[b]).astype(ml_dtypes.bfloat16),
            "wt": wt,
            "gamma": gamma,
            "beta": beta,
        }
        for b in range(B)
    ]
    res = run_bass_kernel_spmd(nc, in_maps, core_ids=list(range(B)), trace=_trace)
    out = np.stack([res.results[b]["out"] for b in range(B)], axis=0)
    if _trace:
        return out, res
    return out


# revision 49
# speedup vs baseline: 1.0502x; 1.0155x over previous
"""ASPPConv (Gaussian-weighted dilated conv1d + sync BatchNorm + ReLU) on 8 Trainium2 cores.

Strategy: data-parallel over batch (B=8 -> 1 batch element per core).

Host prep (layout only): x cast to bf16, coords zero-padded to L+2*PAD,
conv weight pre-transposed to the PE-stationary [c, (k,cc,oc)*128+o] bf16 layout.

Per core:
  phase W: Gaussian tap weights wk[k,l] = exp(-||c(l+dk)-c(l)||^2 / (2 sigma^2))
           for the 8 non-center taps (center tap k=4 has wk identical 1), packed
           [96,1024] per 4096-col chunk (4 col-groups x 8 taps x 3 dims on
           partitions), 96->32 partition-sum via a small PE matmul, exp on ACT,
           stored to DRAM tile-major [NLT, 8, LT] bf16.
  phase M: per 512-col l-tile: DMA the bf16 x chunk into a padded persistent
           buffer, DMA-broadcast wk rows to [128, 8*512], DVE-multiply the 8
           shifted x windows by wk (2x-mode bf16), accumulate 36 bf16 matmuls
           into PSUM [o=128,512] (2 o-chunks; center tap streams x directly),
           evict PSUM->SBUF y (bf16) on ACT with per-channel sum accumulation,
           plus an ACT Square over PSUM accumulating per-channel sum-of-squares.
  phase S: finalize per-channel stats, ncfw AllReduce [128,4] across the 8
           cores (sync BN; a tiny dummy AllReduce early in the kernel warms the
           CC stream), compute scale/shift per channel.
  phase P: ACT Relu(scale*y + bias) / DVE affine+max0 per 2048-col chunk,
           stores fanned across queues.
"""


import numpy as np
import ml_dtypes

import concourse.bass as bass
import concourse.bacc as bacc
import concourse.tile as tile
import concourse.mybir as mybir
from concourse.bass_utils import run_bass_kernel_spmd

F32 = mybir.dt.float32
BF16 = mybir.dt.bfloat16
AF = mybir.ActivationFunctionType

B = 8
CIN = 256
COUT = 256
L = 16384
KT = 9
DIL = 6
PAD = DIL * (KT // 2)  # 24
SIGMA = float(DIL)     # 6.0
INV2S2 = 1.0 / (2.0 * SIGMA * SIGMA)
BN_EPS = 1e-5
NTOT = float(B * L)

LT = 512
NLT = L // LT          # 32
NCC = CIN // 128       # 2
NOC = COUT // 128      # 2
LPADDED = L + 2 * PAD
NTAP = KT - 1          # 8 weighted taps (center tap k=4 has wk == 1)

# W phase packing: partition groups cover contiguous quarters of L; each
# chunk processes 512 cols of every group at once, so chunk ch covers
# l-tiles {g*8 + ch} and the first conv tile only needs one small chunk
WG = 4                 # col-groups (l-quarters) on partitions
WGC = 512              # cols per group per chunk
NWCH = (L // WG) // WGC  # 8 chunks


def _ap(base, extra_offset, free_dims):
    """Custom AP on the same tensor as `base`: keep the partition dim, replace
    the free dims."""
    return bass.AP(
        tensor=base.tensor,
        offset=base.offset + extra_offset,
        ap=[list(base.ap[0])] + [list(d) for d in free_dims],
    )


def _build_program(nc, n_cores):
    x_d = nc.dram_tensor("x", [CIN, L], BF16, kind="ExternalInput")
    # host-gathered coord layouts (pure indexing): partition p = w*48+g*12+kq*3+d,
    # group g covers l in [g*L/4, (g+1)*L/4); cg holds the tap-shifted coords
    # (zero-padded at the ends), cr the center coords
    cg_d = nc.dram_tensor("cg", [96, L // WG], BF16, kind="ExternalInput")
    cr_d = nc.dram_tensor("cr", [96, L // WG], BF16, kind="ExternalInput")
    wt_d = nc.dram_tensor("wt", [128, KT * NCC * NOC * 128], BF16, kind="ExternalInput")
    g_d = nc.dram_tensor("gamma", [COUT], F32, kind="ExternalInput")
    bt_d = nc.dram_tensor("beta", [COUT], F32, kind="ExternalInput")
    o_d = nc.dram_tensor("out", [COUT, L], F32, kind="ExternalOutput")

    # 96->32 partition-sum matrix for the W phase, block structure:
    # partition p = w*48 + g*12 + kq*3 + d  ->  column w*4 + kq + g*8
    # (taps split in two stride-uniform halves w: k in {0..3} and {5..8})
    bmat = np.zeros((2 * WG * 4 * 3, WG * NTAP), dtype=np.float32)
    for w in range(2):
        for g in range(WG):
            for kq in range(4):
                for dd in range(3):
                    bmat[w * 48 + g * 12 + kq * 3 + dd, g * NTAP + w * 4 + kq] = 1.0
    b_dram = nc.inline_tensor(bmat.astype(ml_dtypes.bfloat16), name="bmat")

    from contextlib import ExitStack
    from concourse.tile_rust import add_dep_helper

    with tile.TileContext(nc) as tc, ExitStack() as stk:
        # ---------------- DRAM scratch ----------------
        dram = stk.enter_context(tc.tile_pool(name="dram", bufs=1, space="DRAM"))
        # tap weights, one dram tile per (chunk, group) so each per-tile
        # broadcast read depends only on its own producer scatter:
        # wk_dd[(ch,g)][h, ki, :] covers l-tiles g*8 + ch*2 + h
        wk_dd = {
            ch: dram.tile([WG, NTAP, LT], BF16, tag=f"wk{ch}", name=f"wk{ch}")
            for ch in range(NWCH)
        }
        ccin_d = dram.tile([128, 2 * NOC], F32, tag="ccin", name="ccin")
        ccout_d = dram.tile([128, 2 * NOC], F32, tag="ccout", name="ccout")
        dwi_d = dram.tile([128, 1], F32, tag="dwi", name="dwi")
        dwo_d = dram.tile([128, 1], F32, tag="dwo", name="dwo")

        # ---------------- persistent tiles ----------------
        pers = stk.enter_context(tc.tile_pool(name="pers", bufs=1))
        xbf = [
            pers.tile([128, LPADDED], BF16, tag=f"xbf{cc}", name=f"xbf{cc}")
            for cc in range(NCC)
        ]
        ybf = [
            pers.tile([128, L], BF16, tag=f"ybf{oc}", name=f"ybf{oc}")
            for oc in range(NOC)
        ]
        wT = pers.tile([128, KT * NCC * NOC * 128], BF16, tag="wT", name="wT")
        s1buf = pers.tile([128, NOC, NLT], F32, tag="s1buf", name="s1buf")
        s2buf = pers.tile([128, NOC, NLT], F32, tag="s2buf", name="s2buf")
        scl = pers.tile([128, NOC], F32, tag="scl", name="scl")
        shf = pers.tile([128, NOC], F32, tag="shf", name="shf")

        # warm the CC stream early so the stats AllReduce's setup cost is paid
        # while the conv loop runs (contents of dwi/dwo are don't-care)
        if n_cores > 1:
            nc.gpsimd.collective_compute(
                "AllReduce",
                mybir.AluOpType.add,
                replica_groups=[list(range(n_cores))],
                ins=[dwi_d[:].opt()],
                outs=[dwo_d[:].opt()],
            )

        for cc in range(NCC):
            nc.vector.memset(xbf[cc][:, 0:PAD], 0.0)
            nc.vector.memset(xbf[cc][:, L + PAD : L + 2 * PAD], 0.0)

        # x chunk loads: straight bf16 DMA into the padded persistent buffer
        load_insts = {}

        def load_chunk(t):
            insts = []
            for cc in range(NCC):
                di = nc.gpsimd.dma_start(
                    xbf[cc][:, PAD + t * LT : PAD + (t + 1) * LT],
                    x_d[cc * 128 : (cc + 1) * 128, t * LT : (t + 1) * LT],
                )
                insts.append(di)
            load_insts[t] = insts

        first_loads = []
        load_chunk(0)
        first_loads += load_insts[0]
        load_chunk(1)
        first_loads += load_insts[1]

        # ---------------- phases W+M interleaved ----------------
        sp = stk.enter_context(tc.tile_pool(name="sp", bufs=1))
        with (
            tc.tile_pool(name="mp", bufs=2) as mp,
            tc.tile_pool(name="psm", bufs=2, space="PSUM") as psm,
        ):
            bsb = mp.tile([96, WG * NTAP], BF16, tag="bsb", name="bsb", bufs=1)
            nc.sync.dma_start(bsb[:], b_dram[:, :])
            # coord gather split finest-first: chunk 0 alone (98KB, lands
            # earliest), then chunk 1, then the rest — the chunk-0 chain is
            # the kernel's critical path
            c96 = mp.tile([96, NWCH * WGC], BF16, tag="c96", name="c96", bufs=1)
            crep = mp.tile([96, NWCH * WGC], BF16, tag="crep", name="crep", bufs=1)
            cgi = nc.sync.dma_start(c96[:, 0:WGC], cg_d[:, 0:WGC])
            cri = nc.sync.dma_start(crep[:, 0:WGC], cr_d[:, 0:WGC])
            nc.sync.dma_start(c96[:, WGC : 2 * WGC], cg_d[:, WGC : 2 * WGC])
            nc.sync.dma_start(crep[:, WGC : 2 * WGC], cr_d[:, WGC : 2 * WGC])
            nc.sync.dma_start(c96[:, 2 * WGC :], cg_d[:, 2 * WGC :])
            nc.sync.dma_start(crep[:, 2 * WGC :], cr_d[:, 2 * WGC :])
            # wT is big (1.2MB) and needed a bit later than the coords: keep it
            # out of the startup HBM burst
            wti = nc.gpsimd.dma_start(wT[:], wt_d[:, :])
            add_dep_helper(wti.ins, cgi.ins, reason="defer wT behind coord gather")
            add_dep_helper(wti.ins, cri.ins, reason="defer wT behind coord gather")
            # the first x chunks too: the W chain is the head critical path
            for di in first_loads:
                add_dep_helper(di.ins, cgi.ins, reason="defer x behind coord gather")
                add_dep_helper(di.ins, cri.ins, reason="defer x behind coord gather")

            def emit_w_chunk(ch):
                """Gaussian tap weights for l-tiles {g*8 + ch}."""
                sl = slice(ch * WGC, (ch + 1) * WGC)
                diff = mp.tile([96, WGC], BF16, tag="diff", name="diff", bufs=1)
                nc.vector.tensor_sub(diff[:], c96[:, sl], crep[:, sl])
                sq = mp.tile([96, WGC], BF16, tag="sq", name="sq", bufs=1)
                nc.vector.tensor_mul(sq[:], diff[:], diff[:])
                # 3 bufs: hide the ~2us scatter-DMA completion so the pw chain
                # runs at exp+doorbell cadence (it gates the first conv tile)
                wkch = mp.tile([WG * NTAP, WGC], BF16, tag="wkch", name="wkch", bufs=3)
                pw = psm.tile([WG * NTAP, WGC], F32, tag="pw", name="pw")
                nc.tensor.matmul(pw[:], bsb[:], sq[:])
                nc.scalar.activation(
                    out=wkch[:], in_=pw[:], func=AF.Exp, scale=-INV2S2
                )
                # one contiguous scatter per chunk: wk_dd[ch][g, ki, :]
                nc.scalar.dma_start(wk_dd[ch][:, :, :], wkch[:])

            # all W chunks up front at high priority: the W DVE ops are cheap
            # (16 x ~0.3us) and must not interleave behind xw ops on the DVE
            # FIFO, which would couple the scatter->broadcast round trip into
            # the W pipeline
            with tc.high_priority():
                for ch in range(NWCH):
                    emit_w_chunk(ch)
            xw_insts = {}
            for t in range(NLT):
                if t + 2 < NLT:
                    load_chunk(t + 2)
                    # pace the x prefetch: keep the startup HBM burst small so
                    # the W coord gather isn't starved for bandwidth
                    if t - 2 in xw_insts:
                        for di in load_insts[t + 2]:
                            add_dep_helper(
                                di.ins, xw_insts[t - 2].ins, reason="x prefetch pacing"
                            )
                wkb = mp.tile([128, NTAP * LT], BF16, tag="wkb", name="wkb")
                wkt = wk_dd[t % 8]
                src = bass.AP(
                    tensor=wkt.tensor,
                    offset=wkt.offset + (t // 8) * NTAP * LT,
                    ap=[[0, 128], [1, NTAP * LT]],
                )
                nc.sync.dma_start(wkb[:], src)
                xw = [
                    mp.tile([128, NTAP, LT], BF16, tag=f"xw{cc}", name=f"xw{cc}")
                    for cc in range(NCC)
                ]
                for cc in range(NCC):
                    for w in range(2):
                        src0 = _ap(
                            xbf[cc][:], t * LT + w * 5 * DIL, [[DIL, 4], [1, LT]]
                        )
                        wkv = _ap(wkb[:], w * 4 * LT, [[LT, 4], [1, LT]])
                        tt = nc.vector.tensor_mul(
                            xw[cc][:, w * 4 : (w + 1) * 4, :], src0, wkv
                        )
                        xw_insts[t] = tt
                        # windows read 24 cols into chunk t+1 (and start in
                        # chunk t-1); pin the RAW deps explicitly
                        for dep_t in (t - 1, t, t + 1):
                            for di in load_insts.get(dep_t, []):
                                add_dep_helper(
                                    tt.ins, di.ins, reason="x window chunk overlap"
                                )
                ps = [
                    psm.tile([128, LT], F32, tag=f"ps{oc}", name=f"ps{oc}")
                    for oc in range(NOC)
                ]
                for cc in range(NCC):
                    for k in range(KT):
                        if k == 4:
                            rhs = _ap(xbf[cc][:], t * LT + 4 * DIL, [[1, LT]])
                        else:
                            ki = k if k < 4 else k - 1
                            rhs = xw[cc][:, ki, :]
                        for oc in range(NOC):
                            idx = (k * NCC + cc) * NOC + oc
                            mm = nc.tensor.matmul(
                                ps[oc][:],
                                wT[:, idx * 128 : (idx + 1) * 128],
                                rhs,
                                start=(cc == 0 and k == 0),
                                stop=(cc == NCC - 1 and k == KT - 1),
                            )
                            if k == 4:
                                for di in load_insts.get(t, []):
                                    add_dep_helper(
                                        mm.ins, di.ins, reason="center tap x read"
                                    )
                sqd = psm.tile([128, LT], F32, tag="sqd", name="sqd", bufs=1)
                for oc in range(NOC):
                    nc.scalar.activation(
                        out=ybf[oc][:, t * LT : (t + 1) * LT],
                        in_=ps[oc][:],
                        func=AF.Copy,
                        accum_out=s1buf[:, oc, t : t + 1],
                    )
                    nc.scalar.activation(
                        out=sqd[:],
                        in_=ps[oc][:],
                        func=AF.Square,
                        accum_out=s2buf[:, oc, t : t + 1],
                    )

            # ------------ phase S: stats + sync-BN allreduce ------------
            stats = sp.tile([128, 2 * NOC], F32, tag="stats", name="stats")
            for oc in range(NOC):
                nc.vector.reduce_sum(
                    stats[:, oc : oc + 1], s1buf[:, oc, :], mybir.AxisListType.X
                )
                nc.vector.reduce_sum(
                    stats[:, NOC + oc : NOC + oc + 1],
                    s2buf[:, oc, :],
                    mybir.AxisListType.X,
                )
            allst = sp.tile([128, 2 * NOC], F32, tag="allst", name="allst")
            nc.sync.dma_start(ccin_d[:, :], stats[:])
            if n_cores == 1:
                # timeline-sim mode: no collectives supported; plain copy
                nc.sync.dma_start(ccout_d[:, :], ccin_d[:, :])
            else:
                nc.gpsimd.collective_compute(
                    "AllReduce",
                    mybir.AluOpType.add,
                    replica_groups=[list(range(n_cores))],
                    ins=[ccin_d[:].opt()],
                    outs=[ccout_d[:].opt()],
                )
            nc.sync.dma_start(allst[:], ccout_d[:, :])

            mean = sp.tile([128, NOC], F32, tag="mean", name="mean")
            nc.vector.tensor_scalar_mul(mean[:], allst[:, 0:NOC], 1.0 / NTOT)
            e2 = sp.tile([128, NOC], F32, tag="e2", name="e2")
            nc.vector.tensor_scalar_mul(e2[:], allst[:, NOC : 2 * NOC], 1.0 / NTOT)
            var = sp.tile([128, NOC], F32, tag="var", name="var")
            nc.vector.tensor_mul(var[:], mean[:], mean[:])
            nc.vector.tensor_sub(var[:], e2[:], var[:])
            epsc = sp.tile([128, 1], F32, tag="epsc", name="epsc")
            nc.vector.memset(epsc[:], BN_EPS)
            std = sp.tile([128, NOC], F32, tag="std", name="std")
            nc.scalar.activation(std[:], var[:], func=AF.Sqrt, bias=epsc[:])
            rstd = sp.tile([128, NOC], F32, tag="rstd", name="rstd")
            nc.vector.reciprocal(rstd[:], std[:])

            gsb = sp.tile([128, NOC], F32, tag="gsb", name="gsb")
            nc.sync.dma_start(
                gsb[:], bass.AP(tensor=g_d, offset=0, ap=[[1, 128], [128, NOC]])
            )
            btsb = sp.tile([128, NOC], F32, tag="btsb", name="btsb")
            nc.sync.dma_start(
                btsb[:], bass.AP(tensor=bt_d, offset=0, ap=[[1, 128], [128, NOC]])
            )
            nc.vector.tensor_mul(scl[:], gsb[:], rstd[:])
            tmp = sp.tile([128, NOC], F32, tag="tmp", name="tmp")
            nc.vector.tensor_mul(tmp[:], mean[:], scl[:])
            nc.vector.tensor_sub(shf[:], btsb[:], tmp[:])

        # ---------------- phase P: normalize + relu + store ----------------
        # alternate tiles between ACT (fused relu) and DVE (affine + max0) so
        # the two engines halve the compute; DMAs fan across queues
        # stores only on sync/scalar: gpsimd's teardown DRAIN is slow (~1.7us
        # each) and would gate kernel exit on its last store's completion
        PT = 2 * LT  # two l-tiles per op/DMA (0.5MB stores)
        engs = [nc.sync, nc.scalar]
        with tc.tile_pool(name="pp", bufs=8) as pp:
            i = 0
            for t in range(L // PT):
                for oc in range(NOC):
                    ot = pp.tile([128, PT], F32, tag="ot", name="ot")
                    ysl = ybf[oc][:, t * PT : (t + 1) * PT]
                    if i % 2 == 0:
                        nc.scalar.activation(
                            out=ot[:],
                            in_=ysl,
                            func=AF.Relu,
                            scale=scl[:, oc : oc + 1],
                            bias=shf[:, oc : oc + 1],
                        )
                    else:
                        nc.vector.tensor_scalar(
                            out=ot[:],
                            in0=ysl,
                            scalar1=scl[:, oc : oc + 1],
                            scalar2=shf[:, oc : oc + 1],
                            op0=mybir.AluOpType.mult,
                            op1=mybir.AluOpType.add,
                        )
                        nc.vector.tensor_scalar_max(out=ot[:], in0=ot[:], scalar1=0.0)
                    # each chunk stored as two half-DMAs on different queues
                    # for finer engine balance; gpsimd helps early only, so its
                    # slow teardown DRAIN completes while sync/scalar finish
                    if i < 20:
                        ea = [nc.sync, nc.scalar, nc.gpsimd][i % 3]
                        eb = [nc.scalar, nc.gpsimd, nc.sync][i % 3]
                    else:
                        ea, eb = engs[i % 2], engs[(i + 1) % 2]
                    H = PT // 2
                    ea.dma_start(
                        o_d[oc * 128 : (oc + 1) * 128, t * PT : t * PT + H],
                        ot[:, 0:H],
                    )
                    eb.dma_start(
                        o_d[oc * 128 : (oc + 1) * 128, t * PT + H : (t + 1) * PT],
                        ot[:, H:PT],
                    )
                    i += 1

    return nc


_NC_CACHE = {}


def _get_nc(n_cores=B):
    if n_cores not in _NC_CACHE:
        nc = bacc.Bacc(
            "TRN2", target_bir_lowering=False, debug=False, num_devices=n_cores
        )
        _build_program(nc, n_cores)
        nc.compile()
        _NC_CACHE[n_cores] = nc
    return _NC_CACHE[n_cores]


def _install_ntff_hook():
    """The trimmed image lacks antenv.axon_hooks; synthesize it and register the
    ctypes-based NTFF profile hook so run_bass_kernel_spmd(trace=True) works."""
    import sys
    import types

    if "antenv.axon_hooks" in sys.modules:
        return
    mod = types.ModuleType("antenv.axon_hooks")
    state = {"hook": None}
    mod.set_axon_ntff_profile_hook = lambda h: state.__setitem__("hook", h)
    mod.get_axon_ntff_profile_hook = lambda: state["hook"]
    sys.modules["antenv.axon_hooks"] = mod
    try:
        from trn_agent_boot.trn_boot import _ntff_profile_via_ctypes

        mod.set_axon_ntff_profile_hook(
            _ntff_profile_via_ctypes("/opt/axon/libaxon_pjrt.so")
        )
    except Exception as e:
        print(f"ntff hook install failed: {e}")


def kernel(x, coords, weight, gamma, beta, _trace=False):
    if _trace:
        _install_ntff_hook()
    x = np.ascontiguousarray(x, dtype=np.float32)
    coords = np.ascontiguousarray(coords, dtype=np.float32)
    weight = np.ascontiguousarray(weight, dtype=np.float32)
    gamma = np.ascontiguousarray(gamma, dtype=np.float32)
    beta = np.ascontiguousarray(beta, dtype=np.float32)

    # host layout prep (pure indexing/copies): bf16 x, gathered coord layouts,
    # PE-stationary transposed weight
    x_bf = x.astype(ml_dtypes.bfloat16)
    cpad = np.zeros((B, 3, LPADDED), dtype=np.float32)
    cpad[:, :, PAD : PAD + L] = coords
    LQ = L // WG
    cg = np.empty((B, 96, LQ), dtype=np.float32)
    cr = np.empty((B, 96, LQ), dtype=np.float32)
    # bf16 coords are plenty: |wk error| <= ~0.05% given sigma=6
    for w in range(2):
        for g in range(WG):
            for kq in range(4):
                p = w * 48 + g * 12 + kq * 3
                off = g * LQ + w * 5 * DIL + kq * DIL
                cg[:, p : p + 3, :] = cpad[:, :, off : off + LQ]
                cr[:, p : p + 3, :] = cpad[:, :, PAD + g * LQ : PAD + (g + 1) * LQ]
    # wt[cw, ((k*NCC+cc)*NOC+oc)*128 + ow] = weight[oc*128+ow, cc*128+cw, k]
    wt = np.ascontiguousarray(
        weight.reshape(NOC, 128, NCC, 128, KT)
        .transpose(3, 4, 2, 0, 1)
        .reshape(128, KT * NCC * NOC * 128)
    ).astype(ml_dtypes.bfloat16)

    nc = _get_nc(B)
    in_maps = [
        {
            "x": np.ascontiguousarray(x_bf[b]),
            "cg": np.ascontiguousarray(cg[b]).astype(ml_dtypes.bfloat16),
            "cr": np.ascontiguousarray(cr[b]).astype(ml_dtypes.bfloat16),
            "wt": wt,
            "gamma": gamma,
            "beta": beta,
        }
        for b in range(B)
    ]
    res = run_bass_kernel_spmd(nc, in_maps, core_ids=list(range(B)), trace=_trace)
    out = np.stack([res.results[b]["out"] for b in range(B)], axis=0)
    if _trace:
        return out, res
    return out
